# revision 3
# baseline (speedup 1.0000x reference)
"""CompressedAttention kernel for 8 TRN2 NeuronCores.

Sharding: 2 cores per batch element (core = 2*b + j).
  - compress: core j handles compressed tokens [1024*j, 1024*(j+1)) of its
    batch (bf16 matmuls, fp32 accumulation).
  - importance: core j handles heads [8*j, 8*j+8) (fp32 score matmuls, fp32
    exp via ACT, per-row softmax normalization fused into a running DVE
    accumulation, partition reduction via PE transpose + pairwise tree).
Host: sums the two per-core importance partials, takes top-k (order matters),
and assembles the interleaved output rows with numpy fancy indexing.
"""

import numpy as np
import ml_dtypes

import concourse.bass as bass
import concourse.mybir as mybir
import concourse.tile as ctile
from concourse.tile import TileContext
from concourse.bass_utils import run_bass_kernel_spmd
from concourse.masks import make_identity
from concourse.vector_clock import ScopedClock

B, T_M, C = 4, 4096, 2048
H, KV, T_W, D = 16, 8, 1024, 128
T_CMP = T_M // 2
NUM_SELECTED = 512
OUT_LEN = T_CMP + NUM_SELECTED
SCALING = float(D) ** -0.5
GROUPS = H // KV
TH = T_CMP // 2          # tokens per core (1024)
HH = H // 2              # heads per core (8)
KVH = KV // 2            # kv heads per core (4)
C2 = 2 * C               # 4096
P = 128

BF16 = mybir.dt.bfloat16
F32 = mybir.dt.float32
AF = mybir.ActivationFunctionType


# ---------------------------------------------------------------------------
# Workarounds for this container's walrus build: it rejects more than one
# sync-wait command per instruction, so waits get split onto NoOps.
def _drain_and_barrier_split(self, tick_clock, wait_clock):
    nc = self.nc
    carrier = nc.sync.nop(nofuse=True, hint="tail_wait_carrier")
    wait_clock.add_sem_waits(carrier.ins, ScopedClock({None: tick_clock.global_clock}))
    si = carrier.ins.sync_info
    if si is not None and si.on_wait and len(si.on_wait) > 1:
        extra = list(si.on_wait[1:])
        del si.on_wait[1:]
        for w in extra:
            n2 = nc.sync.nop(nofuse=True, hint="tail_wait_carrier")
            if n2.ins.sync_info is None:
                n2.ins.sync_info = type(si)(on_wait=[w], on_update=[])
            else:
                n2.ins.sync_info.on_wait.append(w)
    nc.sync.drain()
    nc.all_engine_barrier()
    assert self.sems is not None
    popped = nc._tile_sem_poison_stack.pop()
    assert popped is self._sem_poison
    nc.clear_and_free_semaphores(list(self.sems.allocated().values()))
    nc.all_engine_barrier()


def _split_multi_waits(nc, max_waits=1):
    n_split = 0
    for fn in nc.m.functions:
        for blk in fn.blocks:
            insts = blk.instructions
            out = []
            for inst in insts:
                si = getattr(inst, "sync_info", None)
                if si is not None and si.on_wait and len(si.on_wait) > max_waits:
                    extra = list(si.on_wait[max_waits:])
                    del si.on_wait[max_waits:]
                    for j, w in enumerate(extra):
                        nop = mybir.InstNoOp(name=f"{inst.name}_ws{j}", ins=[], outs=[])
                        nop.engine = inst.engine
                        nop.sync_info = mybir.SyncInfo(on_wait=[w], on_update=[])
                        out.append(nop)
                        n_split += 1
                out.append(inst)
            if len(out) != len(insts):
                blk.instructions[:] = out
    return n_split


ctile.TileContext._drain_and_barrier = _drain_and_barrier_split
# ---------------------------------------------------------------------------


def build_nc():
    nc = bass.Bass(target_bir_lowering=False)

    xt_d = nc.dram_tensor("xt", [C2, TH], BF16, kind="ExternalInput")
    wgt_d = nc.dram_tensor("wgt", [C2, C], BF16, kind="ExternalInput")
    wut_d = nc.dram_tensor("wut", [C2, C], BF16, kind="ExternalInput")
    wdt_d = nc.dram_tensor("wdt", [C, C], BF16, kind="ExternalInput")
    qt_d = nc.dram_tensor("qt", [HH * D, T_W], F32, kind="ExternalInput")
    kt_d = nc.dram_tensor("kt", [KVH * D, T_CMP], F32, kind="ExternalInput")

    xmc_d = nc.dram_tensor("xmc", [TH, C], F32, kind="ExternalOutput")
    imp_d = nc.dram_tensor("imp", [P, T_CMP // P], F32, kind="ExternalOutput")

    with TileContext(nc) as tc:
        with tc.tile_pool(name="const", bufs=1) as constp, \
             tc.tile_pool(name="hT", bufs=1) as hTp, \
             tc.tile_pool(name="acc", bufs=1) as accp, \
             tc.tile_pool(name="qk", bufs=2) as qkp, \
             tc.tile_pool(name="esb", bufs=4) as esp, \
             tc.tile_pool(name="small", bufs=4) as smp, \
             tc.tile_pool(name="pss", bufs=2, space="PSUM") as pssp:

            ident = constp.tile([P, P], F32)
            make_identity(nc, ident[:])

            hT = hTp.tile([P, C // P, TH], BF16)          # [128, 16, 1024]
            acc = accp.tile([P, T_CMP], F32)              # [128, 2048]
            nc.vector.memset(acc[:], 0.0)

            # ---------------- phase 1: gate/up -> hT ----------------
            with tc.tile_pool(name="xt", bufs=1) as xtp, \
                 tc.tile_pool(name="wg", bufs=2) as wgp, \
                 tc.tile_pool(name="wu", bufs=2) as wup, \
                 tc.tile_pool(name="sg", bufs=2) as sgp, \
                 tc.tile_pool(name="psg", bufs=1, space="PSUM") as psgp, \
                 tc.tile_pool(name="psu", bufs=1, space="PSUM") as psup:

                xts = xtp.tile([P, C2 // P, TH], BF16)    # [128, 32, 1024]
                nc.sync.dma_start(
                    out=xts[:], in_=xt_d.rearrange("(k p) t -> p k t", p=P))

                for ic in range(C // P):                   # 16 i chunks of 128
                    wg_t = wgp.tile([P, C2 // P, P], BF16, tag="wg")
                    wu_t = wup.tile([P, C2 // P, P], BF16, tag="wu")
                    isl = slice(ic * P, (ic + 1) * P)
                    nc.sync.dma_start(
                        out=wg_t[:],
                        in_=wgt_d[:, isl].rearrange("(k p) i -> p k i", p=P))
                    nc.sync.dma_start(
                        out=wu_t[:],
                        in_=wut_d[:, isl].rearrange("(k p) i -> p k i", p=P))
                    pg = psgp.tile([P, TH], F32, tag="pg")
                    pu = psup.tile([P, TH], F32, tag="pu")
                    for th in range(TH // 512):            # 2 t halves
                        tsl = slice(th * 512, (th + 1) * 512)
                        for k in range(C2 // P):           # 32 k chunks
                            st = (k == 0)
                            sp = (k == C2 // P - 1)
                            nc.tensor.matmul(
                                pg[:, tsl], lhsT=wg_t[:, k, :],
                                rhs=xts[:, k, tsl], start=st, stop=sp)
                        for k in range(C2 // P):
                            st = (k == 0)
                            sp = (k == C2 // P - 1)
                            nc.tensor.matmul(
                                pu[:, tsl], lhsT=wu_t[:, k, :],
                                rhs=xts[:, k, tsl], start=st, stop=sp)
                    sg = sgp.tile([P, TH], F32, tag="sg")
                    nc.scalar.activation(sg[:], pg[:], AF.Silu)
                    nc.vector.tensor_tensor(
                        out=hT[:, ic, :], in0=sg[:], in1=pu[:],
                        op=mybir.AluOpType.mult)

            # ---------------- importance (overlaps compress on PE gaps) ------
            KH = T_CMP // 2                                # 1024 keys per half
            for h in range(HH):
                q_t = qkp.tile([P, T_W], F32, tag="q")
                k_t = qkp.tile([P, T_CMP], F32, tag="k")
                nc.sync.dma_start(out=q_t[:], in_=qt_d[h * D:(h + 1) * D, :])
                kv = h // GROUPS
                nc.sync.dma_start(out=k_t[:], in_=kt_d[kv * D:(kv + 1) * D, :])
                for qi in range(T_W // P):                 # 8 q tiles
                    rs2 = smp.tile([P, 2], F32, tag="rs2")
                    e_half = []
                    for half in range(2):
                        ps = pssp.tile([P, KH], F32, tag="ps")  # 2 banks
                        for n in range(KH // 512):
                            nsl = slice(half * KH + n * 512,
                                        half * KH + (n + 1) * 512)
                            nc.tensor.matmul(
                                ps[:, n * 512:(n + 1) * 512],
                                lhsT=q_t[:, qi * P:(qi + 1) * P],
                                rhs=k_t[:, nsl], start=True, stop=True)
                        e_t = esp.tile([P, KH], F32, tag="e")
                        nc.scalar.activation(
                            e_t[:], ps[:], AF.Exp, scale=SCALING,
                            accum_out=rs2[:, half:half + 1])
                        e_half.append(e_t)
                    rtot = smp.tile([P, 1], F32, tag="rtot")
                    nc.vector.tensor_add(
                        out=rtot[:], in0=rs2[:, 0:1], in1=rs2[:, 1:2])
                    rinv = smp.tile([P, 1], F32, tag="rinv")
                    nc.vector.reciprocal(rinv[:], rtot[:])
                    for half in range(2):
                        asl = slice(half * KH, (half + 1) * KH)
                        nc.vector.scalar_tensor_tensor(
                            out=acc[:, asl], in0=e_half[half][:],
                            scalar=rinv[:, :1], in1=acc[:, asl],
                            op0=mybir.AluOpType.mult, op1=mybir.AluOpType.add)

            # ---------------- phase 2: down -> xmc ----------------
            with tc.tile_pool(name="wd", bufs=1) as wdp, \
                 tc.tile_pool(name="xstage", bufs=2) as xsp, \
                 tc.tile_pool(name="psd", bufs=2, space="PSUM") as psdp:

                wd_t = wdp.tile([P, C // P, C], BF16)      # [128, 16, 2048]
                nc.sync.dma_start(
                    out=wd_t[:], in_=wdt_d.rearrange("(k p) c -> p k c", p=P))

                for m in range(TH // P):                   # 8 token tiles
                    for nh in range(2):                    # 2 c halves of 1024
                        pd = psdp.tile([P, C // 2], F32, tag="pd")  # 2 banks
                        for n in range(2):                 # 512-wide chunks
                            nsl = slice(nh * 1024 + n * 512,
                                        nh * 1024 + (n + 1) * 512)
                            for k in range(C // P):        # 16 i chunks
                                nc.tensor.matmul(
                                    pd[:, n * 512:(n + 1) * 512],
                                    lhsT=hT[:, k, m * P:(m + 1) * P],
                                    rhs=wd_t[:, k, nsl],
                                    start=(k == 0), stop=(k == C // P - 1))
                        xstage = xsp.tile([P, C // 2], F32, tag="xstage")
                        nc.scalar.activation(xstage[:], pd[:], AF.Silu)
                        nc.sync.dma_start(
                            out=xmc_d[m * P:(m + 1) * P,
                                      nh * 1024:(nh + 1) * 1024],
                            in_=xstage[:])

            # ---------------- partition-reduce acc -> imp ----------------
            imp_sb = smp.tile([P, T_CMP // P], F32, tag="impsb")
            for cc in range(T_CMP // P):                   # 16 chunks
                pt = pssp.tile([P, P], F32, tag="ps")
                nc.tensor.transpose(
                    pt[:], acc[:, cc * P:(cc + 1) * P], ident[:])
                tr = esp.tile([P, P], F32, tag="tr")
                nc.scalar.activation(tr[:], pt[:], AF.Copy)
                w = P // 2
                while w >= 1:
                    nc.vector.tensor_add(
                        out=tr[:, :w], in0=tr[:, :w], in1=tr[:, w:2 * w])
                    w //= 2
                nc.vector.tensor_copy(imp_sb[:, cc:cc + 1], tr[:, :1])
            nc.sync.dma_start(out=imp_d[:], in_=imp_sb[:])

    _split_multi_waits(nc)
    return nc


_NC = None


def _get_nc():
    global _NC
    if _NC is None:
        _NC = build_nc()
    return _NC


def kernel(x_m, q_w, km_cmp, W_gate, W_up, W_down):
    x_m = np.asarray(x_m, np.float32)
    q_w = np.asarray(q_w, np.float32)
    km_cmp = np.asarray(km_cmp, np.float32)

    bf = ml_dtypes.bfloat16
    wgt = np.ascontiguousarray(np.asarray(W_gate, np.float32).T).astype(bf)
    wut = np.ascontiguousarray(np.asarray(W_up, np.float32).T).astype(bf)
    wdt = np.ascontiguousarray(np.asarray(W_down, np.float32).T).astype(bf)

    in_maps = []
    for core in range(8):
        b, j = divmod(core, 2)
        xp = x_m[b, 2 * TH * j: 2 * TH * (j + 1)].reshape(TH, C2)
        xt = np.ascontiguousarray(xp.T).astype(bf)
        qt = np.ascontiguousarray(
            q_w[b, HH * j: HH * (j + 1)].transpose(0, 2, 1)).reshape(HH * D, T_W)
        kt = np.ascontiguousarray(
            km_cmp[b, KVH * j: KVH * (j + 1)].transpose(0, 2, 1)).reshape(KVH * D, T_CMP)
        in_maps.append({
            "xt": xt, "wgt": wgt, "wut": wut, "wdt": wdt,
            "qt": qt, "kt": kt,
        })

    nc = _get_nc()
    res = run_bass_kernel_spmd(nc, in_maps, core_ids=list(range(8)), trace=False)

    y = np.zeros((B, OUT_LEN, C), np.float32)
    for b in range(B):
        xm_cmp = np.concatenate(
            [res.results[2 * b]["xmc"], res.results[2 * b + 1]["xmc"]], axis=0)
        imp = (res.results[2 * b]["imp"].astype(np.float64)
               + res.results[2 * b + 1]["imp"].astype(np.float64))
        imp = imp.T.reshape(T_CMP) / H

        sel = np.argsort(-imp, kind="stable")[:NUM_SELECTED]
        mask = np.zeros(T_CMP, bool)
        mask[sel] = True
        sizes = np.where(mask, 2, 1).astype(np.int64)
        start = np.cumsum(sizes) - sizes
        y[b, start[~mask]] = xm_cmp[~mask]
        pos = start[mask]                       # seq-ordered selected slots
        y[b, pos] = x_m[b, 2 * sel]             # value-ordered pairs
        y[b, pos + 1] = x_m[b, 2 * sel + 1]
    return y
